# revision 20
# baseline (speedup 1.0000x reference)
"""Criss-cross (CCNet) sparse attention kernel for Trainium2, 8-core data-parallel.

Problem (hardcoded): B=8, CQ=64, CV=512, H=W=128, fp32 I/O.
Per core: one image.  reference:
    energy_H[i,w,j] = sum_c q[c,i,w] k[c,j,w]   (diag i==j masked -inf)
    energy_W[i,w,j] = sum_c q[c,i,w] k[c,i,j]
    att = softmax(concat(energy_H, energy_W), axis=j)  (256-way per pixel)
    out[c,i,w] = sum_j v[c,j,w] att_H[i,w,j] + sum_j v[c,i,j] att_W[i,w,j]

Kernel strategy (v5) — deferred softmax norm + merged-psum PV + XBAR z:
  - q/k cast fp16 on SWDGE load; energies per row i / col w -> exp(E-40)
    -> UNNORMALIZED att_W[j, i*W+w], att_H[j, w*H+i] bf16; att_H diagonal
    zeroed by (1-I) mask; denominators via basis-matmul psum accumulation.
  - normalization deferred past PV (linearity): recip r[i,w] =
    1/(dnH^T + dnW) -> bf16 -> bounced to a DRAM scratch row, then
    broadcast-loaded (leading stride-0 AP) into R[p, i*W+w] bf16 - a
    partition-replicated recip table.  No att scaling pass at all.
  - v chunk (128 ch) loaded c-major bf16 by one SWDGE cast DMA.
    Row-pass lhsT zg[j, i, c] built per 8-i group by XBAR dma-transpose
    (contiguous cm[:, i, :] planes, SP/ACT HWDGE queues - off the PE).
    Col-pass lhsT u2[j, w, c] needs the strided plane cm[:, :, w] which
    the XBAR cannot read, so u2 stays on PE transposes + psum evacs.
  - PV: both passes accumulate into ONE psum group pg[c, 8i, 128w]
    (2 banks): rows d=0..7 first (start only on d=0 and d=4 - exactly
    one zero-region start per 2KB bank), then 128x2 col matmuls with
    4-i strided within-bank writes (start=False: replace pending-zero
    bytes / accumulate - PE zero-region semantics).
  - single fused evac per group on DVE: out_sb = pg * R-slice, then one
    4KB-run store DMA per group, round-robin over the sync / scalar /
    gpsimd queues (independent DMA channels in this machine model).
"""

import threading

import numpy as np

CQ, CV, H, W = 64, 512, 128, 128
PIX = H * W
B = 8
EXP_BIAS = -40.0
CHUNK = 128
N_CHUNKS = CV // CHUNK
GI = 8                      # i-rows per merged psum group (2 psum banks)
NG = H // GI                # groups per chunk


def build_nc():
    import concourse.mybir as mybir
    import concourse.tile as tile
    from concourse import bacc
    from concourse.masks import make_identity

    f32 = mybir.dt.float32
    bf16 = mybir.dt.bfloat16
    fp16 = mybir.dt.float16
    Exp = mybir.ActivationFunctionType.Exp
    add = mybir.AluOpType.add
    mult = mybir.AluOpType.mult

    nc = bacc.Bacc(None, target_bir_lowering=False)

    with tile.TileContext(nc) as tc:
        with (
            tc.tile_pool(name="dram", bufs=1, space="DRAM") as dram,
            tc.tile_pool(name="attp", bufs=1) as attp,
            tc.tile_pool(name="constp", bufs=1) as constp,
            tc.tile_pool(name="dnp", bufs=1) as dnp,
            tc.tile_pool(name="vp", bufs=2) as vp,
        ):
            q_d = dram.tile((CQ, H, W), f32, kind="ExternalInput", name="q", uniquify=False)
            k_d = dram.tile((CQ, H, W), f32, kind="ExternalInput", name="k", uniquify=False)
            v_d = dram.tile((CV, H, W), f32, kind="ExternalInput", name="v", uniquify=False)
            o_d = dram.tile((CV, H, W), f32, kind="ExternalOutput", name="o", uniquify=False)
            r_scr = dram.tile((PIX,), bf16, name="rscratch")

            # att_W[j, i*W + w] ; att_H[j, w*H + i]  (bf16, UNNORMALIZED)
            att_W = attp.tile([128, PIX], bf16)
            att_H = attp.tile([128, PIX], bf16)


            # v chunk tiles cmaj[c, i, j] bf16: one 128-descriptor full-rate
            # SWDGE cast DMA per chunk
            cm_tiles = []

            def load_cmaj(ck):
                cm = vp.tile([CHUNK, H, W], bf16, name="cm", tag="cm")
                nc.gpsimd.dma_start(cm[:], v_d[ck * CHUNK:(ck + 1) * CHUNK])
                cm_tiles.append(cm)

            # ---- phase 1: energies -> exp -> denominators -> recip
            with tc.tile_pool(name="qkp", bufs=1) as qkp:
                # phase-1-only constants live in qkp so their SBUF frees
                # for the phase-2 R/u2 tiles
                ident = qkp.tile([128, 128], f32, name="ident")
                make_identity(nc, ident[:])
                ident_bf = constp.tile([128, 128], bf16)
                nc.vector.tensor_copy(ident_bf[:], ident[:])
                # maskM4[j, (d, i)] = 0 on j==i diagonal else 1 (4 copies)
                maskM4 = qkp.tile([128, 512], bf16, name="maskM4")
                for d in range(4):
                    nc.vector.tensor_scalar(
                        maskM4[:, d * 128:(d + 1) * 128], ident_bf[:],
                        -1.0, 1.0, op0=mult, op1=add,
                    )
                # zb[j, x] = 1 iff x == 128 (dn basis lhsT: zb[:,128-i:256-i])
                zb = qkp.tile([128, 256], bf16, name="zb")
                nc.vector.memset(zb[:], 0.0)
                nc.vector.memset(zb[:, 128:129], 1.0)
                bias_t = qkp.tile([128, 1], f32, name="bias_t")
                nc.vector.memset(bias_t[:], EXP_BIAS)
                q_sb = qkp.tile([CQ, H, W], fp16)
                k_sb = qkp.tile([CQ, H, W], fp16)
                for ih in range(4):
                    sl = slice(ih * 32, ih * 32 + 32)
                    nc.gpsimd.dma_start(k_sb[:, sl, :], k_d[:, sl, :])
                    nc.gpsimd.dma_start(q_sb[:, sl, :], q_d[:, sl, :])
                load_cmaj(0)
                load_cmaj(1)

                with (
                    tc.tile_pool(name="pse", bufs=2, space="PSUM") as pse,
                    tc.tile_pool(name="psdn", bufs=1, space="PSUM") as psdn,
                ):
                    dnW_ps = [psdn.tile([128, 128], f32, name=f"dnW_ps{a}") for a in range(2)]
                    dnH_ps = [psdn.tile([128, 128], f32, name=f"dnH_ps{a}") for a in range(2)]

                    def dn_rows(i0):
                        for d in range(8):
                            i = i0 + d
                            nc.tensor.matmul(
                                dnW_ps[i % 2][:], lhsT=zb[:, 128 - i:256 - i],
                                rhs=att_W[:, i * W:(i + 1) * W],
                                start=(i < 2), stop=(i >= H - 2),
                            )

                    for i0 in range(0, H, 8):
                        pe = pse.tile([128, 1024], f32, name="pe_row", tag="pe")
                        for d in range(8):
                            i = i0 + d
                            nc.tensor.matmul(
                                pe[:, d * 128:(d + 1) * 128],
                                lhsT=k_sb[:, i, :], rhs=q_sb[:, i, :],
                                start=True, stop=True,
                            )
                        nc.scalar.activation(
                            att_W[:, i0 * W:(i0 + 8) * W], pe[:], Exp, bias=bias_t[:]
                        )
                        # dn one iteration behind: the PE never waits this
                        # iteration's exp
                        if i0 >= 8:
                            dn_rows(i0 - 8)
                    dn_rows(H - 8)
                    def dn_cols(w0):
                        for d in range(8):
                            w = w0 + d
                            nc.tensor.matmul(
                                dnH_ps[w % 2][:], lhsT=zb[:, 128 - w:256 - w],
                                rhs=att_H[:, w * H:(w + 1) * H],
                                start=(w < 2), stop=(w >= H - 2),
                            )

                    for w0 in range(0, W, 8):
                        pe = pse.tile([128, 1024], f32, name="pe_col", tag="pe")
                        for d in range(8):
                            w = w0 + d
                            nc.tensor.matmul(
                                pe[:, d * 128:(d + 1) * 128],
                                lhsT=k_sb[:, :, w], rhs=q_sb[:, :, w],
                                start=True, stop=True,
                            )
                        nc.scalar.activation(
                            att_H[:, w0 * H:(w0 + 8) * H], pe[:], Exp, bias=bias_t[:]
                        )
                        for h2 in range(2):
                            sl = att_H[:, (w0 + 4 * h2) * H:(w0 + 4 * h2 + 4) * H]
                            nc.vector.tensor_tensor(sl, sl, maskM4[:], op=mult)
                        if w0 >= 8:
                            dn_cols(w0 - 8)
                    dn_cols(W - 8)

                    # dn[i, w] = dnW[i, w] + dnH[w, i]^T ; recip -> bf16
                    dnW_sb = qkp.tile([128, 128], f32, name="dnW_sb")
                    nc.vector.tensor_copy(dnW_sb[:], dnW_ps[0][:])
                    nc.vector.tensor_tensor(dnW_sb[:], dnW_sb[:], dnW_ps[1][:], op=add)
                    dnH_sb = qkp.tile([128, 128], f32, name="dnH_sb")
                    nc.vector.tensor_copy(dnH_sb[:], dnH_ps[0][:])
                    nc.vector.tensor_tensor(dnH_sb[:], dnH_sb[:], dnH_ps[1][:], op=add)
                    t2 = pse.tile([128, 128], f32, name="t2", tag="pe")
                    nc.tensor.transpose(t2[:], dnH_sb[:], ident[:])  # [i, w]
                    r_iw = qkp.tile([128, 128], f32, name="r_iw")
                    nc.vector.tensor_tensor(r_iw[:], t2[:], dnW_sb[:], op=add)
                    nc.vector.reciprocal(r_iw[:], r_iw[:])
                    r_bf = dnp.tile([128, 128], bf16)
                    nc.vector.tensor_copy(r_bf[:], r_iw[:])

            # ---- phase 2: R table, v operands, merged-psum PV
            with (
                tc.tile_pool(name="rp", bufs=1) as rp,
                tc.tile_pool(name="u2p", bufs=1) as u2p,
                tc.tile_pool(name="zp", bufs=3) as zp,
                tc.tile_pool(name="outp", bufs=2) as outp,
                tc.tile_pool(name="pst", bufs=2, space="PSUM") as pst,
                tc.tile_pool(name="pgp", bufs=3, space="PSUM") as pgp,
            ):
                # R[p, i*W+w] = recip[i, w] for every partition p, via a DRAM
                # bounce (free in this model) + 4 stride-0 broadcast loads
                nc.sync.dma_start(r_scr[:].rearrange("(i w) -> i w", i=128), r_bf[:])
                R_sb = rp.tile([128, PIX], bf16)
                for qtr in range(4):
                    src = r_scr[qtr * 4096:(qtr + 1) * 4096].unsqueeze(0)
                    nc.sync.dma_start(
                        R_sb[:, qtr * 4096:(qtr + 1) * 4096],
                        src.broadcast_to((128, 4096)),
                    )

                evac_ct = [0]

                def evac_u2(dst, src):
                    # u2 psum evacuations: mostly DVE (bf16 2x); zg evacs go
                    # to ACT separately so the per-group chain avoids queueing
                    # behind the DVE evac-TTs
                    idx = evac_ct[0]
                    evac_ct[0] += 1
                    if idx % 4 == 3:
                        nc.scalar.copy(dst, src)
                    else:
                        nc.vector.tensor_copy(dst, src)

                def evac_zg(dst, src):
                    nc.scalar.copy(dst, src)

                def build_u2_octs(u2, cm, o0, o1):
                    # u2[j, w, c] = v[c0+c, j, w] (col-pass lhsT) - PE path
                    for oct8 in range(o0, o1):
                        ps = pst.tile([128, 1024], bf16, name="ps_t", tag="pst")
                        for t in range(8):
                            x = oct8 * 8 + t
                            nc.tensor.transpose(
                                ps[:, t * 128:(t + 1) * 128], cm[:, :, x], ident_bf[:])
                        evac_u2(u2[:, oct8 * 8:oct8 * 8 + 8, :].rearrange(
                            "j w c -> j (w c)"), ps[:])

                def build_u2(cm):
                    u2 = u2p.tile([128, W, CHUNK], bf16, name="u2")
                    build_u2_octs(u2, cm, 0, 16)
                    return u2

                def build_zg(cm, i0):
                    # zg[j, d, c] = v[c0+c, i0+d, j] (row-pass lhsT):
                    # PE transposes + psum evac (XBAR z + DVE evac + stores
                    # deadlocks the scheduler into a serial latency chain)
                    zg = zp.tile([128, GI, CHUNK], bf16, name="zg")
                    ps = pst.tile([128, 1024], bf16, name="ps_t", tag="pst")
                    for d in range(GI):
                        nc.tensor.transpose(
                            ps[:, d * 128:(d + 1) * 128], cm[:, i0 + d, :], ident_bf[:])
                    evac_zg(zg[:].rearrange("j d c -> j (d c)"), ps[:])
                    return zg

                st_ct = [0]
                pending_stores = []

                def flush_store():
                    # stores are emitted one group late so their evac-done
                    # wait is already satisfied at queue head (a store that
                    # waits in a DMA queue convoys everything behind it).
                    # ACT is compute-loaded, so stores go sync/gpsimd only.
                    dst, src = pending_stores.pop(0)
                    q = (nc.sync,)[0]
                    st_ct[0] += 1
                    q.dma_start(dst, src)

                # flat (chunk, group) schedule with 2-group zg lookahead
                sched = [(ck, g) for ck in range(N_CHUNKS) for g in range(NG)]
                zg_q = [build_zg(cm_tiles[0], 0), build_zg(cm_tiles[0], GI)]
                u2 = build_u2(cm_tiles[0])
                del build_u2

                for idx, (ck, g) in enumerate(sched):
                    cm = cm_tiles[ck]
                    i0 = g * GI
                    u2_pending = None
                    if g == 0:
                        # chunks 0/1 prefetched in phase 1; later loads rotate
                        # into the buffer freed by chunk ck-1
                        if ck >= 1 and ck + 1 < N_CHUNKS:
                            load_cmaj(ck + 1)
                        if ck > 0:
                            # first half of u2 now; second half interleaves
                            # after this group's rows so the PE bridges the
                            # evac latency before the cols need u2
                            u2 = u2p.tile([128, W, CHUNK], bf16, name="u2")
                            build_u2_octs(u2, cm, 0, 8)
                            u2_pending = cm

                    zg = zg_q.pop(0)
                    if idx + 2 < len(sched):
                        nck, ng = sched[idx + 2]
                        zg_q.append(build_zg(cm_tiles[nck], ng * GI))

                    pg = pgp.tile([128, GI, W], f32, name="pg")
                    # rows: one zero-region start per psum bank
                    for d in range(GI):
                        i = i0 + d
                        nc.tensor.matmul(
                            pg[:, d, :], lhsT=zg[:, d, :],
                            rhs=att_W[:, i * W:(i + 1) * W],
                            start=(d % 4 == 0), stop=False,
                            skip_group_check=True,
                        )
                    if u2_pending is not None:
                        build_u2_octs(u2, u2_pending, 8, 16)
                    # cols: strided 4-i within-bank writes
                    for w in range(W):
                        for hb in range(2):
                            nc.tensor.matmul(
                                pg[:, hb * 4:hb * 4 + 4, w],
                                lhsT=u2[:, w, :],
                                rhs=att_H[:, w * H + i0 + hb * 4:
                                          w * H + i0 + hb * 4 + 4],
                                start=False,
                                stop=(w == W - 1),
                                skip_group_check=True,
                            )
                    # fused evac: out = pg * R (deferred softmax norm)
                    out_sb = outp.tile([128, GI, W], f32, name="out_sb")
                    nc.vector.tensor_tensor(
                        out_sb[:].rearrange("c a b -> c (a b)"),
                        pg[:].rearrange("c a b -> c (a b)"),
                        R_sb[:, i0 * W:(i0 + GI) * W], op=mult)
                    pending_stores.append((
                        o_d[ck * CHUNK:(ck + 1) * CHUNK, i0:i0 + GI, :],
                        out_sb[:],
                    ))
                    if len(pending_stores) > 1:
                        flush_store()
                while pending_stores:
                    flush_store()

    nc.compile()
    return nc


_CACHE = {}
_LOCK = threading.Lock()


def _get_nc():
    with _LOCK:
        if "nc" not in _CACHE:
            _CACHE["nc"] = build_nc()
        return _CACHE["nc"]


def kernel(proj_query: np.ndarray, proj_key: np.ndarray, proj_value: np.ndarray,
           trace: bool = False):
    from concourse.bass_utils import run_bass_kernel_spmd

    q = np.ascontiguousarray(np.asarray(proj_query, dtype=np.float32))
    k = np.ascontiguousarray(np.asarray(proj_key, dtype=np.float32))
    v = np.ascontiguousarray(np.asarray(proj_value, dtype=np.float32))
    assert q.shape == (B, CQ, H, W) and v.shape == (B, CV, H, W)

    nc = _get_nc()
    in_maps = [{"q": q[b], "k": k[b], "v": v[b]} for b in range(B)]
    res = run_bass_kernel_spmd(nc, in_maps, core_ids=list(range(B)), trace=trace)
    out = np.stack([res.results[b]["o"] for b in range(B)], axis=0)
    if trace:
        kernel.last_exec_time_ns = res.exec_time_ns
        kernel.last_results = res
    return out


if __name__ == "__main__":
    nc = build_nc()
    print("build ok:", nc)


# revision 23
# speedup vs baseline: 1.0754x; 1.0754x over previous
"""Criss-cross (CCNet) sparse attention kernel for Trainium2, 8-core data-parallel.

Problem (hardcoded): B=8, CQ=64, CV=512, H=W=128, fp32 I/O.
Per core: one image.  reference:
    energy_H[i,w,j] = sum_c q[c,i,w] k[c,j,w]   (diag i==j masked -inf)
    energy_W[i,w,j] = sum_c q[c,i,w] k[c,i,j]
    att = softmax(concat(energy_H, energy_W), axis=j)  (256-way per pixel)
    out[c,i,w] = sum_j v[c,j,w] att_H[i,w,j] + sum_j v[c,i,j] att_W[i,w,j]

Kernel strategy (v5) — deferred softmax norm + merged-psum PV + XBAR z:
  - q/k cast fp16 on SWDGE load; energies per row i / col w -> exp(E-40)
    -> UNNORMALIZED att_W[j, i*W+w], att_H[j, w*H+i] bf16; att_H diagonal
    zeroed by (1-I) mask; denominators via basis-matmul psum accumulation.
  - normalization deferred past PV (linearity): recip r[i,w] =
    1/(dnH^T + dnW) -> bf16 -> bounced to a DRAM scratch row, then
    broadcast-loaded (leading stride-0 AP) into R[p, i*W+w] bf16 - a
    partition-replicated recip table.  No att scaling pass at all.
  - v chunk (128 ch) loaded c-major bf16 by one SWDGE cast DMA.
    Row-pass lhsT zg[j, i, c] built per 8-i group by XBAR dma-transpose
    (contiguous cm[:, i, :] planes, SP/ACT HWDGE queues - off the PE).
    Col-pass lhsT u2[j, w, c] needs the strided plane cm[:, :, w] which
    the XBAR cannot read, so u2 stays on PE transposes + psum evacs.
  - PV: both passes accumulate into ONE psum group pg[c, 8i, 128w]
    (2 banks): rows d=0..7 first (start only on d=0 and d=4 - exactly
    one zero-region start per 2KB bank), then 128x2 col matmuls with
    4-i strided within-bank writes (start=False: replace pending-zero
    bytes / accumulate - PE zero-region semantics).
  - single fused evac per group on DVE: out_sb = pg * R-slice, then one
    4KB-run store DMA per group, round-robin over the sync / scalar /
    gpsimd queues (independent DMA channels in this machine model).
"""

import threading

import numpy as np

CQ, CV, H, W = 64, 512, 128, 128
PIX = H * W
B = 8
EXP_BIAS = -40.0
CHUNK = 128
N_CHUNKS = CV // CHUNK
GI = 8                      # i-rows per merged psum group (2 psum banks)
NG = H // GI                # groups per chunk

# tuning knobs (swept; defaults = best found)
CONFIG = {
    "evac_dve": (1, 3, 4, 6),  # of idx%8: which u2/zg evacs run on DVE
    "zg_act": False,         # zg evacs forced to ACT
    "store_qs": ("sync",),
    "zg_ahead": 2,
    "pgp_bufs": 3,
    "zg_mode": "pe",         # pe | xbar_sync | xbar_scalar
    "load_order": "qk_first",  # qk_first | cm0_mid
}


def build_nc():
    import concourse.mybir as mybir
    import concourse.tile as tile
    from concourse import bacc
    from concourse.masks import make_identity

    f32 = mybir.dt.float32
    bf16 = mybir.dt.bfloat16
    fp16 = mybir.dt.float16
    Exp = mybir.ActivationFunctionType.Exp
    add = mybir.AluOpType.add
    mult = mybir.AluOpType.mult

    nc = bacc.Bacc(None, target_bir_lowering=False)

    with tile.TileContext(nc) as tc:
        with (
            tc.tile_pool(name="dram", bufs=1, space="DRAM") as dram,
            tc.tile_pool(name="attp", bufs=1) as attp,
            tc.tile_pool(name="constp", bufs=1) as constp,
            tc.tile_pool(name="dnp", bufs=1) as dnp,
            tc.tile_pool(name="vp", bufs=2) as vp,
        ):
            q_d = dram.tile((CQ, H, W), f32, kind="ExternalInput", name="q", uniquify=False)
            k_d = dram.tile((CQ, H, W), f32, kind="ExternalInput", name="k", uniquify=False)
            v_d = dram.tile((CV, H, W), f32, kind="ExternalInput", name="v", uniquify=False)
            o_d = dram.tile((CV, H, W), f32, kind="ExternalOutput", name="o", uniquify=False)
            r_scr = dram.tile((PIX,), bf16, name="rscratch")

            # att_W[j, i*W + w] ; att_H[j, w*H + i]  (bf16, UNNORMALIZED)
            att_W = attp.tile([128, PIX], bf16)
            att_H = attp.tile([128, PIX], bf16)


            # v chunk tiles cmaj[c, i, j] bf16: one 128-descriptor full-rate
            # SWDGE cast DMA per chunk
            cm_tiles = []

            def load_cmaj(ck):
                cm = vp.tile([CHUNK, H, W], bf16, name="cm", tag="cm")
                nc.gpsimd.dma_start(cm[:], v_d[ck * CHUNK:(ck + 1) * CHUNK])
                cm_tiles.append(cm)

            # ---- phase 1: energies -> exp -> denominators -> recip
            with tc.tile_pool(name="qkp", bufs=1) as qkp:
                # phase-1-only constants live in qkp so their SBUF frees
                # for the phase-2 R/u2 tiles
                ident = qkp.tile([128, 128], f32, name="ident")
                make_identity(nc, ident[:])
                ident_bf = constp.tile([128, 128], bf16)
                nc.vector.tensor_copy(ident_bf[:], ident[:])
                # maskM4[j, (d, i)] = 0 on j==i diagonal else 1 (4 copies)
                maskM4 = qkp.tile([128, 512], bf16, name="maskM4")
                for d in range(4):
                    nc.vector.tensor_scalar(
                        maskM4[:, d * 128:(d + 1) * 128], ident_bf[:],
                        -1.0, 1.0, op0=mult, op1=add,
                    )
                # zb[j, x] = 1 iff x == 128 (dn basis lhsT: zb[:,128-i:256-i])
                zb = qkp.tile([128, 256], bf16, name="zb")
                nc.vector.memset(zb[:], 0.0)
                nc.vector.memset(zb[:, 128:129], 1.0)
                bias_t = qkp.tile([128, 1], f32, name="bias_t")
                nc.vector.memset(bias_t[:], EXP_BIAS)
                q_sb = qkp.tile([CQ, H, W], fp16)
                k_sb = qkp.tile([CQ, H, W], fp16)
                if CONFIG["load_order"] == "qk_first":
                    for ih in range(4):
                        sl = slice(ih * 32, ih * 32 + 32)
                        nc.gpsimd.dma_start(k_sb[:, sl, :], k_d[:, sl, :])
                        nc.gpsimd.dma_start(q_sb[:, sl, :], q_d[:, sl, :])
                    load_cmaj(0)
                    load_cmaj(1)
                else:
                    for ih in range(2):
                        sl = slice(ih * 32, ih * 32 + 32)
                        nc.gpsimd.dma_start(k_sb[:, sl, :], k_d[:, sl, :])
                        nc.gpsimd.dma_start(q_sb[:, sl, :], q_d[:, sl, :])
                    load_cmaj(0)
                    for ih in range(2, 4):
                        sl = slice(ih * 32, ih * 32 + 32)
                        nc.gpsimd.dma_start(k_sb[:, sl, :], k_d[:, sl, :])
                        nc.gpsimd.dma_start(q_sb[:, sl, :], q_d[:, sl, :])
                    load_cmaj(1)

                with (
                    tc.tile_pool(name="pse", bufs=2, space="PSUM") as pse,
                    tc.tile_pool(name="psdn", bufs=1, space="PSUM") as psdn,
                ):
                    dnW_ps = [psdn.tile([128, 128], f32, name=f"dnW_ps{a}") for a in range(2)]
                    dnH_ps = [psdn.tile([128, 128], f32, name=f"dnH_ps{a}") for a in range(2)]

                    def dn_rows(i0):
                        for d in range(8):
                            i = i0 + d
                            nc.tensor.matmul(
                                dnW_ps[i % 2][:], lhsT=zb[:, 128 - i:256 - i],
                                rhs=att_W[:, i * W:(i + 1) * W],
                                start=(i < 2), stop=(i >= H - 2),
                            )

                    for i0 in range(0, H, 8):
                        pe = pse.tile([128, 1024], f32, name="pe_row", tag="pe")
                        for d in range(8):
                            i = i0 + d
                            nc.tensor.matmul(
                                pe[:, d * 128:(d + 1) * 128],
                                lhsT=k_sb[:, i, :], rhs=q_sb[:, i, :],
                                start=True, stop=True,
                            )
                        nc.scalar.activation(
                            att_W[:, i0 * W:(i0 + 8) * W], pe[:], Exp, bias=bias_t[:]
                        )
                        # dn one iteration behind: the PE never waits this
                        # iteration's exp
                        if i0 >= 8:
                            dn_rows(i0 - 8)
                    dn_rows(H - 8)
                    def dn_cols(w0):
                        for d in range(8):
                            w = w0 + d
                            nc.tensor.matmul(
                                dnH_ps[w % 2][:], lhsT=zb[:, 128 - w:256 - w],
                                rhs=att_H[:, w * H:(w + 1) * H],
                                start=(w < 2), stop=(w >= H - 2),
                            )

                    for w0 in range(0, W, 8):
                        pe = pse.tile([128, 1024], f32, name="pe_col", tag="pe")
                        for d in range(8):
                            w = w0 + d
                            nc.tensor.matmul(
                                pe[:, d * 128:(d + 1) * 128],
                                lhsT=k_sb[:, :, w], rhs=q_sb[:, :, w],
                                start=True, stop=True,
                            )
                        nc.scalar.activation(
                            att_H[:, w0 * H:(w0 + 8) * H], pe[:], Exp, bias=bias_t[:]
                        )
                        for h2 in range(2):
                            sl = att_H[:, (w0 + 4 * h2) * H:(w0 + 4 * h2 + 4) * H]
                            nc.vector.tensor_tensor(sl, sl, maskM4[:], op=mult)
                        if w0 >= 8:
                            dn_cols(w0 - 8)
                    dn_cols(W - 8)

                    # dn[i, w] = dnW[i, w] + dnH[w, i]^T ; recip -> bf16
                    dnW_sb = qkp.tile([128, 128], f32, name="dnW_sb")
                    nc.vector.tensor_copy(dnW_sb[:], dnW_ps[0][:])
                    nc.vector.tensor_tensor(dnW_sb[:], dnW_sb[:], dnW_ps[1][:], op=add)
                    dnH_sb = qkp.tile([128, 128], f32, name="dnH_sb")
                    nc.vector.tensor_copy(dnH_sb[:], dnH_ps[0][:])
                    nc.vector.tensor_tensor(dnH_sb[:], dnH_sb[:], dnH_ps[1][:], op=add)
                    t2 = pse.tile([128, 128], f32, name="t2", tag="pe")
                    nc.tensor.transpose(t2[:], dnH_sb[:], ident[:])  # [i, w]
                    r_iw = qkp.tile([128, 128], f32, name="r_iw")
                    nc.vector.tensor_tensor(r_iw[:], t2[:], dnW_sb[:], op=add)
                    nc.vector.reciprocal(r_iw[:], r_iw[:])
                    r_bf = dnp.tile([128, 128], bf16)
                    nc.vector.tensor_copy(r_bf[:], r_iw[:])

            # ---- phase 2: R table, v operands, merged-psum PV
            with (
                tc.tile_pool(name="rp", bufs=1) as rp,
                tc.tile_pool(name="u2p", bufs=1) as u2p,
                tc.tile_pool(name="zp", bufs=3) as zp,
                tc.tile_pool(name="outp", bufs=2) as outp,
                tc.tile_pool(name="pst", bufs=2, space="PSUM") as pst,
                tc.tile_pool(name="pgp", bufs=CONFIG["pgp_bufs"], space="PSUM") as pgp,
            ):
                # R[p, i*W+w] = recip[i, w] for every partition p, via a DRAM
                # bounce (free in this model) + 4 stride-0 broadcast loads
                nc.sync.dma_start(r_scr[:].rearrange("(i w) -> i w", i=128), r_bf[:])
                R_sb = rp.tile([128, PIX], bf16)
                for qtr in range(4):
                    src = r_scr[qtr * 4096:(qtr + 1) * 4096].unsqueeze(0)
                    nc.sync.dma_start(
                        R_sb[:, qtr * 4096:(qtr + 1) * 4096],
                        src.broadcast_to((128, 4096)),
                    )

                evac_ct = [0]

                def evac_u2(dst, src):
                    idx = evac_ct[0]
                    evac_ct[0] += 1
                    if idx % 8 in CONFIG["evac_dve"]:
                        nc.vector.tensor_copy(dst, src)
                    else:
                        nc.scalar.copy(dst, src)

                def evac_zg(dst, src):
                    if CONFIG["zg_act"]:
                        nc.scalar.copy(dst, src)
                    else:
                        evac_u2(dst, src)

                def build_u2_octs(u2, cm, o0, o1):
                    # u2[j, w, c] = v[c0+c, j, w] (col-pass lhsT) - PE path
                    for oct8 in range(o0, o1):
                        ps = pst.tile([128, 1024], bf16, name="ps_t", tag="pst")
                        for t in range(8):
                            x = oct8 * 8 + t
                            nc.tensor.transpose(
                                ps[:, t * 128:(t + 1) * 128], cm[:, :, x], ident_bf[:])
                        evac_u2(u2[:, oct8 * 8:oct8 * 8 + 8, :].rearrange(
                            "j w c -> j (w c)"), ps[:])

                def build_u2(cm):
                    u2 = u2p.tile([128, W, CHUNK], bf16, name="u2")
                    build_u2_octs(u2, cm, 0, 16)
                    return u2

                def build_zg(cm, i0):
                    # zg[j, d, c] = v[c0+c, i0+d, j] (row-pass lhsT)
                    zg = zp.tile([128, GI, CHUNK], bf16, name="zg")
                    if CONFIG["zg_mode"] == "pe":
                        ps = pst.tile([128, 1024], bf16, name="ps_t", tag="pst")
                        for d in range(GI):
                            nc.tensor.transpose(
                                ps[:, d * 128:(d + 1) * 128], cm[:, i0 + d, :], ident_bf[:])
                        evac_zg(zg[:].rearrange("j d c -> j (d c)"), ps[:])
                    else:
                        q = nc.sync if CONFIG["zg_mode"] == "xbar_sync" else nc.scalar
                        q.dma_start(zg[:], cm[:, i0:i0 + GI, :].rearrange(
                            "c a b -> c (a b)"), transpose=True)
                    return zg

                st_ct = [0]
                pending_stores = []

                def flush_store():
                    # stores are emitted one group late so their evac-done
                    # wait is already satisfied at queue head (a store that
                    # waits in a DMA queue convoys everything behind it).
                    # ACT is compute-loaded, so stores go sync/gpsimd only.
                    dst, src = pending_stores.pop(0)
                    qs = CONFIG["store_qs"]
                    q = getattr(nc, qs[st_ct[0] % len(qs)])
                    st_ct[0] += 1
                    q.dma_start(dst, src)

                # flat (chunk, group) schedule with 2-group zg lookahead
                sched = [(ck, g) for ck in range(N_CHUNKS) for g in range(NG)]
                zg_q = [build_zg(cm_tiles[0], a * GI) for a in range(CONFIG["zg_ahead"])]
                u2 = build_u2(cm_tiles[0])
                del build_u2

                for idx, (ck, g) in enumerate(sched):
                    cm = cm_tiles[ck]
                    i0 = g * GI
                    u2_pending = None
                    if g == 0:
                        # chunks 0/1 prefetched in phase 1; later loads rotate
                        # into the buffer freed by chunk ck-1
                        if ck >= 1 and ck + 1 < N_CHUNKS:
                            load_cmaj(ck + 1)
                        if ck > 0:
                            # first half of u2 now; second half interleaves
                            # after this group's rows so the PE bridges the
                            # evac latency before the cols need u2
                            u2 = u2p.tile([128, W, CHUNK], bf16, name="u2")
                            build_u2_octs(u2, cm, 0, 8)
                            u2_pending = cm

                    zg = zg_q.pop(0)
                    if idx + CONFIG["zg_ahead"] < len(sched):
                        nck, ng = sched[idx + CONFIG["zg_ahead"]]
                        zg_q.append(build_zg(cm_tiles[nck], ng * GI))

                    pg = pgp.tile([128, GI, W], f32, name="pg")
                    # rows: one zero-region start per psum bank
                    for d in range(GI):
                        i = i0 + d
                        nc.tensor.matmul(
                            pg[:, d, :], lhsT=zg[:, d, :],
                            rhs=att_W[:, i * W:(i + 1) * W],
                            start=(d % 4 == 0), stop=False,
                            skip_group_check=True,
                        )
                    if u2_pending is not None:
                        build_u2_octs(u2, u2_pending, 8, 16)
                    # cols: strided 4-i within-bank writes
                    for w in range(W):
                        for hb in range(2):
                            nc.tensor.matmul(
                                pg[:, hb * 4:hb * 4 + 4, w],
                                lhsT=u2[:, w, :],
                                rhs=att_H[:, w * H + i0 + hb * 4:
                                          w * H + i0 + hb * 4 + 4],
                                start=False,
                                stop=(w == W - 1),
                                skip_group_check=True,
                            )
                    # fused evac: out = pg * R (deferred softmax norm)
                    out_sb = outp.tile([128, GI, W], f32, name="out_sb")
                    nc.vector.tensor_tensor(
                        out_sb[:].rearrange("c a b -> c (a b)"),
                        pg[:].rearrange("c a b -> c (a b)"),
                        R_sb[:, i0 * W:(i0 + GI) * W], op=mult)
                    pending_stores.append((
                        o_d[ck * CHUNK:(ck + 1) * CHUNK, i0:i0 + GI, :],
                        out_sb[:],
                    ))
                    if len(pending_stores) > 1:
                        flush_store()
                while pending_stores:
                    flush_store()

    nc.compile()
    return nc


_CACHE = {}
_LOCK = threading.Lock()


def _get_nc():
    with _LOCK:
        if "nc" not in _CACHE:
            _CACHE["nc"] = build_nc()
        return _CACHE["nc"]


def kernel(proj_query: np.ndarray, proj_key: np.ndarray, proj_value: np.ndarray,
           trace: bool = False):
    from concourse.bass_utils import run_bass_kernel_spmd

    q = np.ascontiguousarray(np.asarray(proj_query, dtype=np.float32))
    k = np.ascontiguousarray(np.asarray(proj_key, dtype=np.float32))
    v = np.ascontiguousarray(np.asarray(proj_value, dtype=np.float32))
    assert q.shape == (B, CQ, H, W) and v.shape == (B, CV, H, W)

    nc = _get_nc()
    in_maps = [{"q": q[b], "k": k[b], "v": v[b]} for b in range(B)]
    res = run_bass_kernel_spmd(nc, in_maps, core_ids=list(range(B)), trace=trace)
    out = np.stack([res.results[b]["o"] for b in range(B)], axis=0)
    if trace:
        kernel.last_exec_time_ns = res.exec_time_ns
        kernel.last_results = res
    return out


if __name__ == "__main__":
    nc = build_nc()
    print("build ok:", nc)


# revision 25
# speedup vs baseline: 1.0804x; 1.0047x over previous
"""Criss-cross (CCNet) sparse attention kernel for Trainium2, 8-core data-parallel.

Problem (hardcoded): B=8, CQ=64, CV=512, H=W=128, fp32 I/O.
Per core: one image.  reference:
    energy_H[i,w,j] = sum_c q[c,i,w] k[c,j,w]   (diag i==j masked -inf)
    energy_W[i,w,j] = sum_c q[c,i,w] k[c,i,j]
    att = softmax(concat(energy_H, energy_W), axis=j)  (256-way per pixel)
    out[c,i,w] = sum_j v[c,j,w] att_H[i,w,j] + sum_j v[c,i,j] att_W[i,w,j]

Kernel strategy (v6) — deferred softmax normalization + merged-psum PV:
  - q/k cast fp16 on SWDGE load (32-row slices so energies start early);
    energies per row i / col w -> exp(E-40) -> UNNORMALIZED
    att_W[j, i*W+w], att_H[j, w*H+i] bf16; att_H diagonal zeroed by a
    (1-I) mask; denominators via basis-matmul psum accumulation, with
    the dn matmuls one exp-iteration behind so the PE never waits ACT.
  - normalization deferred past PV (linearity): recip r[i,w] =
    1/(dnH^T + dnW) -> bf16 -> bounced to a DRAM scratch row, then
    broadcast-loaded (leading stride-0 AP over the DRAM side) into
    R[p, i*W+w] bf16, a partition-replicated recip table.  The whole
    baseline att-scaling phase (r2 flats, rank-1 psum broadcasts, DVE
    multiplies, ACT restages) disappears.
  - v chunk (128 ch) loaded c-major bf16 by one SWDGE cast DMA (cast
    loads are charged on the bf16 side).  Both PV operand layouts are
    built by PE transposes + ACT/DVE psum evacs: u2[j, w, c] per chunk,
    zg[j, i, c] per 8-i group just-in-time, two groups ahead.  (XBAR
    dma-transpose builds of zg sim correct but convoy the scheduler's
    DMA-semaphore chains - measured 1.5-2x slower end-to-end.)
  - PV: both passes accumulate into ONE psum group pg[c, 8i, 128w]
    (2 banks): rows d=0..7 first (start=True only on d=0 and d=4 -
    exactly one zero-region start per 2KB bank), then 128x2 col matmuls
    with 4-i strided within-bank writes (start=False: replace
    pending-zero bytes / accumulate - PE zero-region semantics).  This
    replaces the baseline's separate col-psum ACT copy + row-psum DVE
    add with nothing.
  - single fused evac per group on DVE: out_sb = pg * R-slice, then one
    4KB-run store DMA per group on the sync queue, emitted one group
    late so its wait is pre-satisfied at queue head.
"""

import threading

import numpy as np

CQ, CV, H, W = 64, 512, 128, 128
PIX = H * W
B = 8
EXP_BIAS = -40.0
CHUNK = 128
N_CHUNKS = CV // CHUNK
GI = 8                      # i-rows per merged psum group (2 psum banks)
NG = H // GI                # groups per chunk

# tuning knobs (swept; defaults = best found)
CONFIG = {
    "evac_dve": (0, 1, 3, 4, 6),  # of idx%8: which u2/zg evacs run on DVE
    "zg_act": False,         # zg evacs forced to ACT
    "store_qs": ("sync",),
    "zg_ahead": 2,
    "pgp_bufs": 3,
    "zg_mode": "pe",         # pe | xbar_sync | xbar_scalar
    "load_order": "qk_first",  # qk_first | cm0_mid
}


def build_nc():
    import concourse.mybir as mybir
    import concourse.tile as tile
    from concourse import bacc
    from concourse.masks import make_identity

    f32 = mybir.dt.float32
    bf16 = mybir.dt.bfloat16
    fp16 = mybir.dt.float16
    Exp = mybir.ActivationFunctionType.Exp
    add = mybir.AluOpType.add
    mult = mybir.AluOpType.mult

    nc = bacc.Bacc(None, target_bir_lowering=False)

    with tile.TileContext(nc) as tc:
        with (
            tc.tile_pool(name="dram", bufs=1, space="DRAM") as dram,
            tc.tile_pool(name="attp", bufs=1) as attp,
            tc.tile_pool(name="constp", bufs=1) as constp,
            tc.tile_pool(name="dnp", bufs=1) as dnp,
            tc.tile_pool(name="vp", bufs=2) as vp,
        ):
            q_d = dram.tile((CQ, H, W), f32, kind="ExternalInput", name="q", uniquify=False)
            k_d = dram.tile((CQ, H, W), f32, kind="ExternalInput", name="k", uniquify=False)
            v_d = dram.tile((CV, H, W), f32, kind="ExternalInput", name="v", uniquify=False)
            o_d = dram.tile((CV, H, W), f32, kind="ExternalOutput", name="o", uniquify=False)
            r_scr = dram.tile((PIX,), bf16, name="rscratch")

            # att_W[j, i*W + w] ; att_H[j, w*H + i]  (bf16, UNNORMALIZED)
            att_W = attp.tile([128, PIX], bf16)
            att_H = attp.tile([128, PIX], bf16)


            # v chunk tiles cmaj[c, i, j] bf16: one 128-descriptor full-rate
            # SWDGE cast DMA per chunk
            cm_tiles = []

            def load_cmaj(ck):
                cm = vp.tile([CHUNK, H, W], bf16, name="cm", tag="cm")
                nc.gpsimd.dma_start(cm[:], v_d[ck * CHUNK:(ck + 1) * CHUNK])
                cm_tiles.append(cm)

            # ---- phase 1: energies -> exp -> denominators -> recip
            with tc.tile_pool(name="qkp", bufs=1) as qkp:
                # phase-1-only constants live in qkp so their SBUF frees
                # for the phase-2 R/u2 tiles
                ident = qkp.tile([128, 128], f32, name="ident")
                make_identity(nc, ident[:])
                ident_bf = constp.tile([128, 128], bf16)
                nc.vector.tensor_copy(ident_bf[:], ident[:])
                # maskM4[j, (d, i)] = 0 on j==i diagonal else 1 (4 copies)
                maskM4 = qkp.tile([128, 512], bf16, name="maskM4")
                for d in range(4):
                    nc.vector.tensor_scalar(
                        maskM4[:, d * 128:(d + 1) * 128], ident_bf[:],
                        -1.0, 1.0, op0=mult, op1=add,
                    )
                # zb[j, x] = 1 iff x == 128 (dn basis lhsT: zb[:,128-i:256-i])
                zb = qkp.tile([128, 256], bf16, name="zb")
                nc.vector.memset(zb[:], 0.0)
                nc.vector.memset(zb[:, 128:129], 1.0)
                bias_t = qkp.tile([128, 1], f32, name="bias_t")
                nc.vector.memset(bias_t[:], EXP_BIAS)
                q_sb = qkp.tile([CQ, H, W], fp16)
                k_sb = qkp.tile([CQ, H, W], fp16)
                if CONFIG["load_order"] == "qk_first":
                    for ih in range(4):
                        sl = slice(ih * 32, ih * 32 + 32)
                        nc.gpsimd.dma_start(k_sb[:, sl, :], k_d[:, sl, :])
                        nc.gpsimd.dma_start(q_sb[:, sl, :], q_d[:, sl, :])
                    load_cmaj(0)
                    load_cmaj(1)
                else:
                    for ih in range(2):
                        sl = slice(ih * 32, ih * 32 + 32)
                        nc.gpsimd.dma_start(k_sb[:, sl, :], k_d[:, sl, :])
                        nc.gpsimd.dma_start(q_sb[:, sl, :], q_d[:, sl, :])
                    load_cmaj(0)
                    for ih in range(2, 4):
                        sl = slice(ih * 32, ih * 32 + 32)
                        nc.gpsimd.dma_start(k_sb[:, sl, :], k_d[:, sl, :])
                        nc.gpsimd.dma_start(q_sb[:, sl, :], q_d[:, sl, :])
                    load_cmaj(1)

                with (
                    tc.tile_pool(name="pse", bufs=2, space="PSUM") as pse,
                    tc.tile_pool(name="psdn", bufs=1, space="PSUM") as psdn,
                ):
                    dnW_ps = [psdn.tile([128, 128], f32, name=f"dnW_ps{a}") for a in range(2)]
                    dnH_ps = [psdn.tile([128, 128], f32, name=f"dnH_ps{a}") for a in range(2)]

                    def dn_rows(i0):
                        for d in range(8):
                            i = i0 + d
                            nc.tensor.matmul(
                                dnW_ps[i % 2][:], lhsT=zb[:, 128 - i:256 - i],
                                rhs=att_W[:, i * W:(i + 1) * W],
                                start=(i < 2), stop=(i >= H - 2),
                            )

                    for i0 in range(0, H, 8):
                        pe = pse.tile([128, 1024], f32, name="pe_row", tag="pe")
                        for d in range(8):
                            i = i0 + d
                            nc.tensor.matmul(
                                pe[:, d * 128:(d + 1) * 128],
                                lhsT=k_sb[:, i, :], rhs=q_sb[:, i, :],
                                start=True, stop=True,
                            )
                        nc.scalar.activation(
                            att_W[:, i0 * W:(i0 + 8) * W], pe[:], Exp, bias=bias_t[:]
                        )
                        # dn one iteration behind: the PE never waits this
                        # iteration's exp
                        if i0 >= 8:
                            dn_rows(i0 - 8)
                    dn_rows(H - 8)
                    def dn_cols(w0):
                        for d in range(8):
                            w = w0 + d
                            nc.tensor.matmul(
                                dnH_ps[w % 2][:], lhsT=zb[:, 128 - w:256 - w],
                                rhs=att_H[:, w * H:(w + 1) * H],
                                start=(w < 2), stop=(w >= H - 2),
                            )

                    for w0 in range(0, W, 8):
                        pe = pse.tile([128, 1024], f32, name="pe_col", tag="pe")
                        for d in range(8):
                            w = w0 + d
                            nc.tensor.matmul(
                                pe[:, d * 128:(d + 1) * 128],
                                lhsT=k_sb[:, :, w], rhs=q_sb[:, :, w],
                                start=True, stop=True,
                            )
                        nc.scalar.activation(
                            att_H[:, w0 * H:(w0 + 8) * H], pe[:], Exp, bias=bias_t[:]
                        )
                        for h2 in range(2):
                            sl = att_H[:, (w0 + 4 * h2) * H:(w0 + 4 * h2 + 4) * H]
                            nc.vector.tensor_tensor(sl, sl, maskM4[:], op=mult)
                        if w0 >= 8:
                            dn_cols(w0 - 8)
                    dn_cols(W - 8)

                    # dn[i, w] = dnW[i, w] + dnH[w, i]^T ; recip -> bf16
                    dnW_sb = qkp.tile([128, 128], f32, name="dnW_sb")
                    nc.vector.tensor_copy(dnW_sb[:], dnW_ps[0][:])
                    nc.vector.tensor_tensor(dnW_sb[:], dnW_sb[:], dnW_ps[1][:], op=add)
                    dnH_sb = qkp.tile([128, 128], f32, name="dnH_sb")
                    nc.vector.tensor_copy(dnH_sb[:], dnH_ps[0][:])
                    nc.vector.tensor_tensor(dnH_sb[:], dnH_sb[:], dnH_ps[1][:], op=add)
                    t2 = pse.tile([128, 128], f32, name="t2", tag="pe")
                    nc.tensor.transpose(t2[:], dnH_sb[:], ident[:])  # [i, w]
                    r_iw = qkp.tile([128, 128], f32, name="r_iw")
                    nc.vector.tensor_tensor(r_iw[:], t2[:], dnW_sb[:], op=add)
                    nc.vector.reciprocal(r_iw[:], r_iw[:])
                    r_bf = dnp.tile([128, 128], bf16)
                    nc.vector.tensor_copy(r_bf[:], r_iw[:])

            # ---- phase 2: R table, v operands, merged-psum PV
            with (
                tc.tile_pool(name="rp", bufs=1) as rp,
                tc.tile_pool(name="u2p", bufs=1) as u2p,
                tc.tile_pool(name="zp", bufs=3) as zp,
                tc.tile_pool(name="outp", bufs=2) as outp,
                tc.tile_pool(name="pst", bufs=2, space="PSUM") as pst,
                tc.tile_pool(name="pgp", bufs=CONFIG["pgp_bufs"], space="PSUM") as pgp,
            ):
                # R[p, i*W+w] = recip[i, w] for every partition p, via a DRAM
                # bounce (free in this model) + 4 stride-0 broadcast loads
                nc.sync.dma_start(r_scr[:].rearrange("(i w) -> i w", i=128), r_bf[:])
                R_sb = rp.tile([128, PIX], bf16)
                for qtr in range(4):
                    src = r_scr[qtr * 4096:(qtr + 1) * 4096].unsqueeze(0)
                    nc.sync.dma_start(
                        R_sb[:, qtr * 4096:(qtr + 1) * 4096],
                        src.broadcast_to((128, 4096)),
                    )

                evac_ct = [0]

                def evac_u2(dst, src):
                    idx = evac_ct[0]
                    evac_ct[0] += 1
                    if idx % 8 in CONFIG["evac_dve"]:
                        nc.vector.tensor_copy(dst, src)
                    else:
                        nc.scalar.copy(dst, src)

                def evac_zg(dst, src):
                    if CONFIG["zg_act"]:
                        nc.scalar.copy(dst, src)
                    else:
                        evac_u2(dst, src)

                def build_u2_octs(u2, cm, o0, o1):
                    # u2[j, w, c] = v[c0+c, j, w] (col-pass lhsT) - PE path
                    for oct8 in range(o0, o1):
                        ps = pst.tile([128, 1024], bf16, name="ps_t", tag="pst")
                        for t in range(8):
                            x = oct8 * 8 + t
                            nc.tensor.transpose(
                                ps[:, t * 128:(t + 1) * 128], cm[:, :, x], ident_bf[:])
                        evac_u2(u2[:, oct8 * 8:oct8 * 8 + 8, :].rearrange(
                            "j w c -> j (w c)"), ps[:])

                def build_u2(cm):
                    u2 = u2p.tile([128, W, CHUNK], bf16, name="u2")
                    build_u2_octs(u2, cm, 0, 16)
                    return u2

                def build_zg(cm, i0):
                    # zg[j, d, c] = v[c0+c, i0+d, j] (row-pass lhsT)
                    zg = zp.tile([128, GI, CHUNK], bf16, name="zg")
                    if CONFIG["zg_mode"] == "pe":
                        ps = pst.tile([128, 1024], bf16, name="ps_t", tag="pst")
                        for d in range(GI):
                            nc.tensor.transpose(
                                ps[:, d * 128:(d + 1) * 128], cm[:, i0 + d, :], ident_bf[:])
                        evac_zg(zg[:].rearrange("j d c -> j (d c)"), ps[:])
                    else:
                        q = nc.sync if CONFIG["zg_mode"] == "xbar_sync" else nc.scalar
                        q.dma_start(zg[:], cm[:, i0:i0 + GI, :].rearrange(
                            "c a b -> c (a b)"), transpose=True)
                    return zg

                st_ct = [0]
                pending_stores = []

                def flush_store():
                    # stores are emitted one group late so their evac-done
                    # wait is already satisfied at queue head (a store that
                    # waits in a DMA queue convoys everything behind it).
                    # ACT is compute-loaded, so stores go sync/gpsimd only.
                    dst, src = pending_stores.pop(0)
                    qs = CONFIG["store_qs"]
                    q = getattr(nc, qs[st_ct[0] % len(qs)])
                    st_ct[0] += 1
                    q.dma_start(dst, src)

                # flat (chunk, group) schedule with 2-group zg lookahead
                sched = [(ck, g) for ck in range(N_CHUNKS) for g in range(NG)]
                zg_q = [build_zg(cm_tiles[0], a * GI) for a in range(CONFIG["zg_ahead"])]
                u2 = build_u2(cm_tiles[0])
                del build_u2

                for idx, (ck, g) in enumerate(sched):
                    cm = cm_tiles[ck]
                    i0 = g * GI
                    u2_pending = None
                    if g == 0:
                        # chunks 0/1 prefetched in phase 1; later loads rotate
                        # into the buffer freed by chunk ck-1
                        if ck >= 1 and ck + 1 < N_CHUNKS:
                            load_cmaj(ck + 1)
                        if ck > 0:
                            # first half of u2 now; second half interleaves
                            # after this group's rows so the PE bridges the
                            # evac latency before the cols need u2
                            u2 = u2p.tile([128, W, CHUNK], bf16, name="u2")
                            build_u2_octs(u2, cm, 0, 8)
                            u2_pending = cm

                    zg = zg_q.pop(0)
                    if idx + CONFIG["zg_ahead"] < len(sched):
                        nck, ng = sched[idx + CONFIG["zg_ahead"]]
                        zg_q.append(build_zg(cm_tiles[nck], ng * GI))

                    pg = pgp.tile([128, GI, W], f32, name="pg")
                    # rows: one zero-region start per psum bank
                    for d in range(GI):
                        i = i0 + d
                        nc.tensor.matmul(
                            pg[:, d, :], lhsT=zg[:, d, :],
                            rhs=att_W[:, i * W:(i + 1) * W],
                            start=(d % 4 == 0), stop=False,
                            skip_group_check=True,
                        )
                    if u2_pending is not None:
                        build_u2_octs(u2, u2_pending, 8, 16)
                    # cols: strided 4-i within-bank writes
                    for w in range(W):
                        for hb in range(2):
                            nc.tensor.matmul(
                                pg[:, hb * 4:hb * 4 + 4, w],
                                lhsT=u2[:, w, :],
                                rhs=att_H[:, w * H + i0 + hb * 4:
                                          w * H + i0 + hb * 4 + 4],
                                start=False,
                                stop=(w == W - 1),
                                skip_group_check=True,
                            )
                    # fused evac: out = pg * R (deferred softmax norm)
                    out_sb = outp.tile([128, GI, W], f32, name="out_sb")
                    nc.vector.tensor_tensor(
                        out_sb[:].rearrange("c a b -> c (a b)"),
                        pg[:].rearrange("c a b -> c (a b)"),
                        R_sb[:, i0 * W:(i0 + GI) * W], op=mult)
                    pending_stores.append((
                        o_d[ck * CHUNK:(ck + 1) * CHUNK, i0:i0 + GI, :],
                        out_sb[:],
                    ))
                    if len(pending_stores) > CONFIG.get("store_delay", 1):
                        flush_store()
                while pending_stores:
                    flush_store()

    nc.compile()
    return nc


_CACHE = {}
_LOCK = threading.Lock()


def _get_nc():
    with _LOCK:
        if "nc" not in _CACHE:
            _CACHE["nc"] = build_nc()
        return _CACHE["nc"]


def kernel(proj_query: np.ndarray, proj_key: np.ndarray, proj_value: np.ndarray,
           trace: bool = False):
    from concourse.bass_utils import run_bass_kernel_spmd

    q = np.ascontiguousarray(np.asarray(proj_query, dtype=np.float32))
    k = np.ascontiguousarray(np.asarray(proj_key, dtype=np.float32))
    v = np.ascontiguousarray(np.asarray(proj_value, dtype=np.float32))
    assert q.shape == (B, CQ, H, W) and v.shape == (B, CV, H, W)

    nc = _get_nc()
    in_maps = [{"q": q[b], "k": k[b], "v": v[b]} for b in range(B)]
    res = run_bass_kernel_spmd(nc, in_maps, core_ids=list(range(B)), trace=trace)
    out = np.stack([res.results[b]["o"] for b in range(B)], axis=0)
    if trace:
        kernel.last_exec_time_ns = res.exec_time_ns
        kernel.last_results = res
    return out


if __name__ == "__main__":
    nc = build_nc()
    print("build ok:", nc)


# revision 27
# speedup vs baseline: 1.2186x; 1.1279x over previous
"""Criss-cross (CCNet) sparse attention kernel for Trainium2, 8-core data-parallel.

Problem (hardcoded): B=8, CQ=64, CV=512, H=W=128, fp32 I/O.
Per core: one image.  reference:
    energy_H[i,w,j] = sum_c q[c,i,w] k[c,j,w]   (diag i==j masked -inf)
    energy_W[i,w,j] = sum_c q[c,i,w] k[c,i,j]
    att = softmax(concat(energy_H, energy_W), axis=j)  (256-way per pixel)
    out[c,i,w] = sum_j v[c,j,w] att_H[i,w,j] + sum_j v[c,i,j] att_W[i,w,j]

Kernel strategy (v6) — deferred softmax normalization + merged-psum PV:
  - q/k cast fp16 on SWDGE load (32-row slices so energies start early);
    energies per row i / col w -> exp(E-40) -> UNNORMALIZED
    att_W[j, i*W+w], att_H[j, w*H+i] bf16; att_H diagonal zeroed by a
    (1-I) mask; denominators via basis-matmul psum accumulation, with
    the dn matmuls one exp-iteration behind so the PE never waits ACT.
  - normalization deferred past PV (linearity): recip r[i,w] =
    1/(dnH^T + dnW) -> bf16 -> bounced to a DRAM scratch row, then
    broadcast-loaded (leading stride-0 AP over the DRAM side) into
    R[p, i*W+w] bf16, a partition-replicated recip table.  The whole
    baseline att-scaling phase (r2 flats, rank-1 psum broadcasts, DVE
    multiplies, ACT restages) disappears.
  - v chunk (128 ch) loaded c-major bf16 by one SWDGE cast DMA (cast
    loads are charged on the bf16 side).  Both PV operand layouts are
    built by PE transposes + ACT/DVE psum evacs: u2[j, w, c] per chunk,
    zg[j, i, c] per 8-i group just-in-time, two groups ahead.  (XBAR
    dma-transpose builds of zg sim correct but convoy the scheduler's
    DMA-semaphore chains - measured 1.5-2x slower end-to-end.)
  - PV: both passes accumulate into ONE psum group pg[c, 8i, 128w]
    (2 banks): rows d=0..7 first (start=True only on d=0 and d=4 -
    exactly one zero-region start per 2KB bank), then 128x2 col matmuls
    with 4-i strided within-bank writes (start=False: replace
    pending-zero bytes / accumulate - PE zero-region semantics).  This
    replaces the baseline's separate col-psum ACT copy + row-psum DVE
    add with nothing.
  - single fused evac per group on DVE: out_sb = pg * R-slice, then one
    4KB-run store DMA per group on the sync queue, emitted one group
    late so its wait is pre-satisfied at queue head.
"""

import threading

import numpy as np

CQ, CV, H, W = 64, 512, 128, 128
PIX = H * W
B = 8
EXP_BIAS = -40.0
CHUNK = 128
N_CHUNKS = CV // CHUNK
GI = 8                      # i-rows per merged psum group (2 psum banks)
NG = H // GI                # groups per chunk

# tuning knobs (swept; defaults = best found)
CONFIG = {
    "evac_dve": (1, 3, 5, 7),  # of idx%8: which u2/zg evacs run on DVE
    "zg_act": False,           # zg evacs forced to ACT
    "store_qs": ("scalar", "sync"),
    "zg_ahead": 2,
    "pgp_bufs": 2,
    "pst_bufs": 4,
    "zg_mode": "pe",           # pe | xbar_sync | xbar_scalar
    "load_order": "qk_first",  # qk_first | cm0_mid
}


def build_nc():
    import concourse.mybir as mybir
    import concourse.tile as tile
    from concourse import bacc
    from concourse.masks import make_identity

    f32 = mybir.dt.float32
    bf16 = mybir.dt.bfloat16
    fp16 = mybir.dt.float16
    Exp = mybir.ActivationFunctionType.Exp
    add = mybir.AluOpType.add
    mult = mybir.AluOpType.mult

    nc = bacc.Bacc(None, target_bir_lowering=False)

    with tile.TileContext(nc) as tc:
        with (
            tc.tile_pool(name="dram", bufs=1, space="DRAM") as dram,
            tc.tile_pool(name="attp", bufs=1) as attp,
            tc.tile_pool(name="constp", bufs=1) as constp,
            tc.tile_pool(name="dnp", bufs=1) as dnp,
            tc.tile_pool(name="vp", bufs=2) as vp,
        ):
            q_d = dram.tile((CQ, H, W), f32, kind="ExternalInput", name="q", uniquify=False)
            k_d = dram.tile((CQ, H, W), f32, kind="ExternalInput", name="k", uniquify=False)
            v_d = dram.tile((CV, H, W), f32, kind="ExternalInput", name="v", uniquify=False)
            o_d = dram.tile((CV, H, W), f32, kind="ExternalOutput", name="o", uniquify=False)
            r_scr = dram.tile((PIX,), bf16, name="rscratch")

            # att_W[j, i*W + w] ; att_H[j, w*H + i]  (bf16, UNNORMALIZED)
            att_W = attp.tile([128, PIX], bf16)
            att_H = attp.tile([128, PIX], bf16)


            # v chunk tiles cmaj[c, i, j] bf16: one 128-descriptor full-rate
            # SWDGE cast DMA per chunk
            cm_tiles = []

            def load_cmaj(ck):
                cm = vp.tile([CHUNK, H, W], bf16, name="cm", tag="cm")
                nc.gpsimd.dma_start(cm[:], v_d[ck * CHUNK:(ck + 1) * CHUNK])
                cm_tiles.append(cm)

            # ---- phase 1: energies -> exp -> denominators -> recip
            with tc.tile_pool(name="qkp", bufs=1) as qkp:
                # phase-1-only constants live in qkp so their SBUF frees
                # for the phase-2 R/u2 tiles
                ident = qkp.tile([128, 128], f32, name="ident")
                make_identity(nc, ident[:])
                ident_bf = constp.tile([128, 128], bf16)
                nc.vector.tensor_copy(ident_bf[:], ident[:])
                # maskM4[j, (d, i)] = 0 on j==i diagonal else 1 (4 copies)
                maskM4 = qkp.tile([128, 512], bf16, name="maskM4")
                for d in range(4):
                    nc.vector.tensor_scalar(
                        maskM4[:, d * 128:(d + 1) * 128], ident_bf[:],
                        -1.0, 1.0, op0=mult, op1=add,
                    )
                # zb[j, x] = 1 iff x == 128 (dn basis lhsT: zb[:,128-i:256-i])
                zb = qkp.tile([128, 256], bf16, name="zb")
                nc.vector.memset(zb[:], 0.0)
                nc.vector.memset(zb[:, 128:129], 1.0)
                bias_t = qkp.tile([128, 1], f32, name="bias_t")
                nc.vector.memset(bias_t[:], EXP_BIAS)
                q_sb = qkp.tile([CQ, H, W], fp16)
                k_sb = qkp.tile([CQ, H, W], fp16)
                if CONFIG["load_order"] == "qk_first":
                    for ih in range(4):
                        sl = slice(ih * 32, ih * 32 + 32)
                        nc.gpsimd.dma_start(k_sb[:, sl, :], k_d[:, sl, :])
                        nc.gpsimd.dma_start(q_sb[:, sl, :], q_d[:, sl, :])
                    load_cmaj(0)
                    load_cmaj(1)
                else:
                    for ih in range(2):
                        sl = slice(ih * 32, ih * 32 + 32)
                        nc.gpsimd.dma_start(k_sb[:, sl, :], k_d[:, sl, :])
                        nc.gpsimd.dma_start(q_sb[:, sl, :], q_d[:, sl, :])
                    load_cmaj(0)
                    for ih in range(2, 4):
                        sl = slice(ih * 32, ih * 32 + 32)
                        nc.gpsimd.dma_start(k_sb[:, sl, :], k_d[:, sl, :])
                        nc.gpsimd.dma_start(q_sb[:, sl, :], q_d[:, sl, :])
                    load_cmaj(1)

                with (
                    tc.tile_pool(name="pse", bufs=2, space="PSUM") as pse,
                    tc.tile_pool(name="psdn", bufs=1, space="PSUM") as psdn,
                ):
                    dnW_ps = [psdn.tile([128, 128], f32, name=f"dnW_ps{a}") for a in range(2)]
                    dnH_ps = [psdn.tile([128, 128], f32, name=f"dnH_ps{a}") for a in range(2)]

                    def dn_rows(i0):
                        for d in range(8):
                            i = i0 + d
                            nc.tensor.matmul(
                                dnW_ps[i % 2][:], lhsT=zb[:, 128 - i:256 - i],
                                rhs=att_W[:, i * W:(i + 1) * W],
                                start=(i < 2), stop=(i >= H - 2),
                            )

                    for i0 in range(0, H, 8):
                        pe = pse.tile([128, 1024], f32, name="pe_row", tag="pe")
                        for d in range(8):
                            i = i0 + d
                            nc.tensor.matmul(
                                pe[:, d * 128:(d + 1) * 128],
                                lhsT=k_sb[:, i, :], rhs=q_sb[:, i, :],
                                start=True, stop=True,
                            )
                        nc.scalar.activation(
                            att_W[:, i0 * W:(i0 + 8) * W], pe[:], Exp, bias=bias_t[:]
                        )
                        # dn one iteration behind: the PE never waits this
                        # iteration's exp
                        if i0 >= 8:
                            dn_rows(i0 - 8)
                    dn_rows(H - 8)
                    def dn_cols(w0):
                        for d in range(8):
                            w = w0 + d
                            nc.tensor.matmul(
                                dnH_ps[w % 2][:], lhsT=zb[:, 128 - w:256 - w],
                                rhs=att_H[:, w * H:(w + 1) * H],
                                start=(w < 2), stop=(w >= H - 2),
                            )

                    for w0 in range(0, W, 8):
                        pe = pse.tile([128, 1024], f32, name="pe_col", tag="pe")
                        for d in range(8):
                            w = w0 + d
                            nc.tensor.matmul(
                                pe[:, d * 128:(d + 1) * 128],
                                lhsT=k_sb[:, :, w], rhs=q_sb[:, :, w],
                                start=True, stop=True,
                            )
                        nc.scalar.activation(
                            att_H[:, w0 * H:(w0 + 8) * H], pe[:], Exp, bias=bias_t[:]
                        )
                        for h2 in range(2):
                            sl = att_H[:, (w0 + 4 * h2) * H:(w0 + 4 * h2 + 4) * H]
                            nc.vector.tensor_tensor(sl, sl, maskM4[:], op=mult)
                        if w0 >= 8:
                            dn_cols(w0 - 8)
                    dn_cols(W - 8)

                    # dn[i, w] = dnW[i, w] + dnH[w, i]^T ; recip -> bf16
                    dnW_sb = qkp.tile([128, 128], f32, name="dnW_sb")
                    nc.vector.tensor_copy(dnW_sb[:], dnW_ps[0][:])
                    nc.vector.tensor_tensor(dnW_sb[:], dnW_sb[:], dnW_ps[1][:], op=add)
                    dnH_sb = qkp.tile([128, 128], f32, name="dnH_sb")
                    nc.vector.tensor_copy(dnH_sb[:], dnH_ps[0][:])
                    nc.vector.tensor_tensor(dnH_sb[:], dnH_sb[:], dnH_ps[1][:], op=add)
                    t2 = pse.tile([128, 128], f32, name="t2", tag="pe")
                    nc.tensor.transpose(t2[:], dnH_sb[:], ident[:])  # [i, w]
                    r_iw = qkp.tile([128, 128], f32, name="r_iw")
                    nc.vector.tensor_tensor(r_iw[:], t2[:], dnW_sb[:], op=add)
                    nc.vector.reciprocal(r_iw[:], r_iw[:])
                    r_bf = dnp.tile([128, 128], bf16)
                    nc.vector.tensor_copy(r_bf[:], r_iw[:])

            # ---- phase 2: R table, v operands, merged-psum PV
            with (
                tc.tile_pool(name="rp", bufs=1) as rp,
                tc.tile_pool(name="u2p", bufs=1) as u2p,
                tc.tile_pool(name="zp", bufs=3) as zp,
                tc.tile_pool(name="outp", bufs=2) as outp,
                tc.tile_pool(name="pst", bufs=CONFIG.get("pst_bufs", 2), space="PSUM") as pst,
                tc.tile_pool(name="pgp", bufs=CONFIG["pgp_bufs"], space="PSUM") as pgp,
            ):
                # R[p, i*W+w] = recip[i, w] for every partition p, via a DRAM
                # bounce (free in this model) + 4 stride-0 broadcast loads
                nc.sync.dma_start(r_scr[:].rearrange("(i w) -> i w", i=128), r_bf[:])
                R_sb = rp.tile([128, PIX], bf16)
                for qtr in range(4):
                    src = r_scr[qtr * 4096:(qtr + 1) * 4096].unsqueeze(0)
                    nc.sync.dma_start(
                        R_sb[:, qtr * 4096:(qtr + 1) * 4096],
                        src.broadcast_to((128, 4096)),
                    )

                evac_ct = [0]

                def evac_u2(dst, src):
                    idx = evac_ct[0]
                    evac_ct[0] += 1
                    if idx % 8 in CONFIG["evac_dve"]:
                        nc.vector.tensor_copy(dst, src)
                    else:
                        nc.scalar.copy(dst, src)

                def evac_zg(dst, src):
                    if CONFIG["zg_act"]:
                        nc.scalar.copy(dst, src)
                    else:
                        evac_u2(dst, src)

                def build_u2_octs(u2, cm, o0, o1):
                    # u2[j, w, c] = v[c0+c, j, w] (col-pass lhsT) - PE path
                    for oct8 in range(o0, o1):
                        ps = pst.tile([128, 1024], bf16, name="ps_t", tag="pst")
                        for t in range(8):
                            x = oct8 * 8 + t
                            nc.tensor.transpose(
                                ps[:, t * 128:(t + 1) * 128], cm[:, :, x], ident_bf[:])
                        evac_u2(u2[:, oct8 * 8:oct8 * 8 + 8, :].rearrange(
                            "j w c -> j (w c)"), ps[:])

                def build_u2(cm):
                    u2 = u2p.tile([128, W, CHUNK], bf16, name="u2")
                    build_u2_octs(u2, cm, 0, 16)
                    return u2

                def build_zg(cm, i0):
                    # zg[j, d, c] = v[c0+c, i0+d, j] (row-pass lhsT)
                    zg = zp.tile([128, GI, CHUNK], bf16, name="zg")
                    if CONFIG["zg_mode"] == "pe":
                        ps = pst.tile([128, 1024], bf16, name="ps_t", tag="pst")
                        for d in range(GI):
                            nc.tensor.transpose(
                                ps[:, d * 128:(d + 1) * 128], cm[:, i0 + d, :], ident_bf[:])
                        evac_zg(zg[:].rearrange("j d c -> j (d c)"), ps[:])
                    else:
                        q = nc.sync if CONFIG["zg_mode"] == "xbar_sync" else nc.scalar
                        q.dma_start(zg[:], cm[:, i0:i0 + GI, :].rearrange(
                            "c a b -> c (a b)"), transpose=True)
                    return zg

                st_ct = [0]
                pending_stores = []

                def flush_store():
                    # stores are emitted one group late so their evac-done
                    # wait is already satisfied at queue head (a store that
                    # waits in a DMA queue convoys everything behind it).
                    # ACT is compute-loaded, so stores go sync/gpsimd only.
                    dst, src = pending_stores.pop(0)
                    qs = CONFIG["store_qs"]
                    q = getattr(nc, qs[st_ct[0] % len(qs)])
                    st_ct[0] += 1
                    q.dma_start(dst, src)

                # flat (chunk, group) schedule with 2-group zg lookahead
                sched = [(ck, g) for ck in range(N_CHUNKS) for g in range(NG)]
                zg_q = [build_zg(cm_tiles[0], a * GI) for a in range(CONFIG["zg_ahead"])]
                u2 = build_u2(cm_tiles[0])
                del build_u2

                for idx, (ck, g) in enumerate(sched):
                    cm = cm_tiles[ck]
                    i0 = g * GI
                    u2_pending = None
                    if g == 0:
                        # chunks 0/1 prefetched in phase 1; later loads rotate
                        # into the buffer freed by chunk ck-1
                        if ck >= 1 and ck + 1 < N_CHUNKS:
                            load_cmaj(ck + 1)
                        if ck > 0:
                            # first half of u2 now; second half interleaves
                            # after this group's rows so the PE bridges the
                            # evac latency before the cols need u2
                            u2 = u2p.tile([128, W, CHUNK], bf16, name="u2")
                            build_u2_octs(u2, cm, 0, 8)
                            u2_pending = cm

                    zg = zg_q.pop(0)
                    if idx + CONFIG["zg_ahead"] < len(sched):
                        nck, ng = sched[idx + CONFIG["zg_ahead"]]
                        zg_q.append(build_zg(cm_tiles[nck], ng * GI))

                    pg = pgp.tile([128, GI, W], f32, name="pg")
                    # rows: one zero-region start per psum bank
                    for d in range(GI):
                        i = i0 + d
                        nc.tensor.matmul(
                            pg[:, d, :], lhsT=zg[:, d, :],
                            rhs=att_W[:, i * W:(i + 1) * W],
                            start=(d % 4 == 0), stop=False,
                            skip_group_check=True,
                        )
                    if u2_pending is not None:
                        build_u2_octs(u2, u2_pending, 8, 16)
                    # cols: strided 4-i within-bank writes
                    for w in range(W):
                        for hb in range(2):
                            nc.tensor.matmul(
                                pg[:, hb * 4:hb * 4 + 4, w],
                                lhsT=u2[:, w, :],
                                rhs=att_H[:, w * H + i0 + hb * 4:
                                          w * H + i0 + hb * 4 + 4],
                                start=False,
                                stop=(w == W - 1),
                                skip_group_check=True,
                            )
                    # fused evac: out = pg * R (deferred softmax norm)
                    out_sb = outp.tile([128, GI, W], f32, name="out_sb")
                    nc.vector.tensor_tensor(
                        out_sb[:].rearrange("c a b -> c (a b)"),
                        pg[:].rearrange("c a b -> c (a b)"),
                        R_sb[:, i0 * W:(i0 + GI) * W], op=mult)
                    pending_stores.append((
                        o_d[ck * CHUNK:(ck + 1) * CHUNK, i0:i0 + GI, :],
                        out_sb[:],
                    ))
                    if len(pending_stores) > CONFIG.get("store_delay", 1):
                        flush_store()
                while pending_stores:
                    flush_store()

    nc.compile()
    return nc


_CACHE = {}
_LOCK = threading.Lock()


def _get_nc():
    with _LOCK:
        if "nc" not in _CACHE:
            _CACHE["nc"] = build_nc()
        return _CACHE["nc"]


def kernel(proj_query: np.ndarray, proj_key: np.ndarray, proj_value: np.ndarray,
           trace: bool = False):
    from concourse.bass_utils import run_bass_kernel_spmd

    q = np.ascontiguousarray(np.asarray(proj_query, dtype=np.float32))
    k = np.ascontiguousarray(np.asarray(proj_key, dtype=np.float32))
    v = np.ascontiguousarray(np.asarray(proj_value, dtype=np.float32))
    assert q.shape == (B, CQ, H, W) and v.shape == (B, CV, H, W)

    nc = _get_nc()
    in_maps = [{"q": q[b], "k": k[b], "v": v[b]} for b in range(B)]
    res = run_bass_kernel_spmd(nc, in_maps, core_ids=list(range(B)), trace=trace)
    out = np.stack([res.results[b]["o"] for b in range(B)], axis=0)
    if trace:
        kernel.last_exec_time_ns = res.exec_time_ns
        kernel.last_results = res
    return out


if __name__ == "__main__":
    nc = build_nc()
    print("build ok:", nc)


# revision 29
# speedup vs baseline: 1.2405x; 1.0180x over previous
"""Criss-cross (CCNet) sparse attention kernel for Trainium2, 8-core data-parallel.

Problem (hardcoded): B=8, CQ=64, CV=512, H=W=128, fp32 I/O.
Per core: one image.  reference:
    energy_H[i,w,j] = sum_c q[c,i,w] k[c,j,w]   (diag i==j masked -inf)
    energy_W[i,w,j] = sum_c q[c,i,w] k[c,i,j]
    att = softmax(concat(energy_H, energy_W), axis=j)  (256-way per pixel)
    out[c,i,w] = sum_j v[c,j,w] att_H[i,w,j] + sum_j v[c,i,j] att_W[i,w,j]

Kernel strategy (v6) — deferred softmax normalization + merged-psum PV:
  - q/k cast fp16 on SWDGE load (32-row slices so energies start early);
    energies per row i / col w -> exp(E-40) -> UNNORMALIZED
    att_W[j, i*W+w], att_H[j, w*H+i] bf16; att_H diagonal zeroed by a
    (1-I) mask; denominators via basis-matmul psum accumulation, with
    the dn matmuls one exp-iteration behind so the PE never waits ACT.
  - normalization deferred past PV (linearity): recip r[i,w] =
    1/(dnH^T + dnW) -> bf16 -> bounced to a DRAM scratch row, then
    broadcast-loaded (leading stride-0 AP over the DRAM side) into
    R[p, i*W+w] bf16, a partition-replicated recip table.  The whole
    baseline att-scaling phase (r2 flats, rank-1 psum broadcasts, DVE
    multiplies, ACT restages) disappears.
  - v chunk (128 ch) loaded c-major bf16 by one SWDGE cast DMA (cast
    loads are charged on the bf16 side).  Both PV operand layouts are
    built by PE transposes + ACT/DVE psum evacs: u2[j, w, c] per chunk,
    zg[j, i, c] per 8-i group just-in-time, two groups ahead.  (XBAR
    dma-transpose builds of zg sim correct but convoy the scheduler's
    DMA-semaphore chains - measured 1.5-2x slower end-to-end.)
  - PV: both passes accumulate into ONE psum group pg[c, 8i, 128w]
    (2 banks): rows d=0..7 first (start=True only on d=0 and d=4 -
    exactly one zero-region start per 2KB bank), then 128x2 col matmuls
    with 4-i strided within-bank writes (start=False: replace
    pending-zero bytes / accumulate - PE zero-region semantics).  This
    replaces the baseline's separate col-psum ACT copy + row-psum DVE
    add with nothing.
  - single fused evac per group on DVE: out_sb = pg * R-slice, then one
    4KB-run store DMA per group on the sync queue, emitted one group
    late so its wait is pre-satisfied at queue head.
"""

import threading

import numpy as np

CQ, CV, H, W = 64, 512, 128, 128
PIX = H * W
B = 8
EXP_BIAS = -40.0
CHUNK = 128
N_CHUNKS = CV // CHUNK
GI = 8                      # i-rows per merged psum group (2 psum banks)
NG = H // GI                # groups per chunk

# tuning knobs (swept; defaults = best found)
CONFIG = {
    "evac_dve": (1, 2, 3, 5, 7),  # of idx%8: which u2/zg evacs run on DVE
    "zg_act": False,              # zg evacs forced to ACT
    "store_qs": ("scalar", "sync"),
    "zg_ahead": 2,
    "pgp_bufs": 2,
    "pst_bufs": 4,
    "qk_slices": (16,) * 8,       # fine q/k slices: energies start ~4.5us
    "zg_mode": "pe",              # pe | xbar_sync | xbar_scalar
    "load_order": "qk_first",     # qk_first | cm0_mid
}


def build_nc():
    import concourse.mybir as mybir
    import concourse.tile as tile
    from concourse import bacc
    from concourse.masks import make_identity

    f32 = mybir.dt.float32
    bf16 = mybir.dt.bfloat16
    fp16 = mybir.dt.float16
    Exp = mybir.ActivationFunctionType.Exp
    add = mybir.AluOpType.add
    mult = mybir.AluOpType.mult

    nc = bacc.Bacc(None, target_bir_lowering=False)

    with tile.TileContext(nc) as tc:
        with (
            tc.tile_pool(name="dram", bufs=1, space="DRAM") as dram,
            tc.tile_pool(name="attp", bufs=1) as attp,
            tc.tile_pool(name="constp", bufs=1) as constp,
            tc.tile_pool(name="dnp", bufs=1) as dnp,
            tc.tile_pool(name="vp", bufs=2) as vp,
        ):
            q_d = dram.tile((CQ, H, W), f32, kind="ExternalInput", name="q", uniquify=False)
            k_d = dram.tile((CQ, H, W), f32, kind="ExternalInput", name="k", uniquify=False)
            v_d = dram.tile((CV, H, W), f32, kind="ExternalInput", name="v", uniquify=False)
            o_d = dram.tile((CV, H, W), f32, kind="ExternalOutput", name="o", uniquify=False)
            r_scr = dram.tile((PIX,), bf16, name="rscratch")

            # att_W[j, i*W + w] ; att_H[j, w*H + i]  (bf16, UNNORMALIZED)
            att_W = attp.tile([128, PIX], bf16)
            att_H = attp.tile([128, PIX], bf16)


            # v chunk tiles cmaj[c, i, j] bf16: one 128-descriptor full-rate
            # SWDGE cast DMA per chunk
            cm_tiles = []

            def load_cmaj(ck):
                cm = vp.tile([CHUNK, H, W], bf16, name="cm", tag="cm")
                nc.gpsimd.dma_start(cm[:], v_d[ck * CHUNK:(ck + 1) * CHUNK])
                cm_tiles.append(cm)

            # ---- phase 1: energies -> exp -> denominators -> recip
            with tc.tile_pool(name="qkp", bufs=1) as qkp:
                # phase-1-only constants live in qkp so their SBUF frees
                # for the phase-2 R/u2 tiles
                ident = qkp.tile([128, 128], f32, name="ident")
                make_identity(nc, ident[:])
                ident_bf = constp.tile([128, 128], bf16)
                nc.vector.tensor_copy(ident_bf[:], ident[:])
                # maskM4[j, (d, i)] = 0 on j==i diagonal else 1 (4 copies)
                maskM4 = qkp.tile([128, 512], bf16, name="maskM4")
                for d in range(4):
                    nc.vector.tensor_scalar(
                        maskM4[:, d * 128:(d + 1) * 128], ident_bf[:],
                        -1.0, 1.0, op0=mult, op1=add,
                    )
                # zb[j, x] = 1 iff x == 128 (dn basis lhsT: zb[:,128-i:256-i])
                zb = qkp.tile([128, 256], bf16, name="zb")
                nc.vector.memset(zb[:], 0.0)
                nc.vector.memset(zb[:, 128:129], 1.0)
                bias_t = qkp.tile([128, 1], f32, name="bias_t")
                nc.vector.memset(bias_t[:], EXP_BIAS)
                q_sb = qkp.tile([CQ, H, W], fp16)
                k_sb = qkp.tile([CQ, H, W], fp16)
                if CONFIG["load_order"] == "qk_first":
                    slices = CONFIG.get("qk_slices", (32, 32, 32, 32))
                    lo = 0
                    for ln in slices:
                        sl = slice(lo, lo + ln)
                        nc.gpsimd.dma_start(k_sb[:, sl, :], k_d[:, sl, :])
                        nc.gpsimd.dma_start(q_sb[:, sl, :], q_d[:, sl, :])
                        lo += ln
                    load_cmaj(0)
                    load_cmaj(1)
                else:
                    for ih in range(2):
                        sl = slice(ih * 32, ih * 32 + 32)
                        nc.gpsimd.dma_start(k_sb[:, sl, :], k_d[:, sl, :])
                        nc.gpsimd.dma_start(q_sb[:, sl, :], q_d[:, sl, :])
                    load_cmaj(0)
                    for ih in range(2, 4):
                        sl = slice(ih * 32, ih * 32 + 32)
                        nc.gpsimd.dma_start(k_sb[:, sl, :], k_d[:, sl, :])
                        nc.gpsimd.dma_start(q_sb[:, sl, :], q_d[:, sl, :])
                    load_cmaj(1)

                with (
                    tc.tile_pool(name="pse", bufs=2, space="PSUM") as pse,
                    tc.tile_pool(name="psdn", bufs=1, space="PSUM") as psdn,
                ):
                    dnW_ps = [psdn.tile([128, 128], f32, name=f"dnW_ps{a}") for a in range(2)]
                    dnH_ps = [psdn.tile([128, 128], f32, name=f"dnH_ps{a}") for a in range(2)]

                    def dn_rows(i0):
                        for d in range(8):
                            i = i0 + d
                            nc.tensor.matmul(
                                dnW_ps[i % 2][:], lhsT=zb[:, 128 - i:256 - i],
                                rhs=att_W[:, i * W:(i + 1) * W],
                                start=(i < 2), stop=(i >= H - 2),
                            )

                    for i0 in range(0, H, 8):
                        pe = pse.tile([128, 1024], f32, name="pe_row", tag="pe")
                        for d in range(8):
                            i = i0 + d
                            nc.tensor.matmul(
                                pe[:, d * 128:(d + 1) * 128],
                                lhsT=k_sb[:, i, :], rhs=q_sb[:, i, :],
                                start=True, stop=True,
                            )
                        nc.scalar.activation(
                            att_W[:, i0 * W:(i0 + 8) * W], pe[:], Exp, bias=bias_t[:]
                        )
                        # dn one iteration behind: the PE never waits this
                        # iteration's exp
                        if i0 >= 8:
                            dn_rows(i0 - 8)
                    dn_rows(H - 8)
                    def dn_cols(w0):
                        for d in range(8):
                            w = w0 + d
                            nc.tensor.matmul(
                                dnH_ps[w % 2][:], lhsT=zb[:, 128 - w:256 - w],
                                rhs=att_H[:, w * H:(w + 1) * H],
                                start=(w < 2), stop=(w >= H - 2),
                            )

                    for w0 in range(0, W, 8):
                        pe = pse.tile([128, 1024], f32, name="pe_col", tag="pe")
                        for d in range(8):
                            w = w0 + d
                            nc.tensor.matmul(
                                pe[:, d * 128:(d + 1) * 128],
                                lhsT=k_sb[:, :, w], rhs=q_sb[:, :, w],
                                start=True, stop=True,
                            )
                        nc.scalar.activation(
                            att_H[:, w0 * H:(w0 + 8) * H], pe[:], Exp, bias=bias_t[:]
                        )
                        for h2 in range(2):
                            sl = att_H[:, (w0 + 4 * h2) * H:(w0 + 4 * h2 + 4) * H]
                            nc.vector.tensor_tensor(sl, sl, maskM4[:], op=mult)
                        if w0 >= 8:
                            dn_cols(w0 - 8)
                    dn_cols(W - 8)

                    # dn[i, w] = dnW[i, w] + dnH[w, i]^T ; recip -> bf16
                    dnW_sb = qkp.tile([128, 128], f32, name="dnW_sb")
                    nc.vector.tensor_copy(dnW_sb[:], dnW_ps[0][:])
                    nc.vector.tensor_tensor(dnW_sb[:], dnW_sb[:], dnW_ps[1][:], op=add)
                    dnH_sb = qkp.tile([128, 128], f32, name="dnH_sb")
                    nc.vector.tensor_copy(dnH_sb[:], dnH_ps[0][:])
                    nc.vector.tensor_tensor(dnH_sb[:], dnH_sb[:], dnH_ps[1][:], op=add)
                    t2 = pse.tile([128, 128], f32, name="t2", tag="pe")
                    nc.tensor.transpose(t2[:], dnH_sb[:], ident[:])  # [i, w]
                    r_iw = qkp.tile([128, 128], f32, name="r_iw")
                    nc.vector.tensor_tensor(r_iw[:], t2[:], dnW_sb[:], op=add)
                    nc.vector.reciprocal(r_iw[:], r_iw[:])
                    r_bf = dnp.tile([128, 128], bf16)
                    nc.vector.tensor_copy(r_bf[:], r_iw[:])

            # ---- phase 2: R table, v operands, merged-psum PV
            with (
                tc.tile_pool(name="rp", bufs=1) as rp,
                tc.tile_pool(name="u2p", bufs=1) as u2p,
                tc.tile_pool(name="zp", bufs=3) as zp,
                tc.tile_pool(name="outp", bufs=2) as outp,
                tc.tile_pool(name="pst", bufs=CONFIG.get("pst_bufs", 2), space="PSUM") as pst,
                tc.tile_pool(name="pgp", bufs=CONFIG["pgp_bufs"], space="PSUM") as pgp,
            ):
                # R[p, i*W+w] = recip[i, w] for every partition p, via a DRAM
                # bounce (free in this model) + 4 stride-0 broadcast loads
                nc.sync.dma_start(r_scr[:].rearrange("(i w) -> i w", i=128), r_bf[:])
                R_sb = rp.tile([128, PIX], bf16)
                for qtr in range(4):
                    src = r_scr[qtr * 4096:(qtr + 1) * 4096].unsqueeze(0)
                    nc.sync.dma_start(
                        R_sb[:, qtr * 4096:(qtr + 1) * 4096],
                        src.broadcast_to((128, 4096)),
                    )

                evac_ct = [0]

                def evac_u2(dst, src):
                    idx = evac_ct[0]
                    evac_ct[0] += 1
                    if idx % 8 in CONFIG["evac_dve"]:
                        nc.vector.tensor_copy(dst, src)
                    else:
                        nc.scalar.copy(dst, src)

                def evac_zg(dst, src):
                    if CONFIG["zg_act"]:
                        nc.scalar.copy(dst, src)
                    else:
                        evac_u2(dst, src)

                def build_u2_octs(u2, cm, o0, o1):
                    # u2[j, w, c] = v[c0+c, j, w] (col-pass lhsT) - PE path
                    for oct8 in range(o0, o1):
                        ps = pst.tile([128, 1024], bf16, name="ps_t", tag="pst")
                        for t in range(8):
                            x = oct8 * 8 + t
                            nc.tensor.transpose(
                                ps[:, t * 128:(t + 1) * 128], cm[:, :, x], ident_bf[:])
                        evac_u2(u2[:, oct8 * 8:oct8 * 8 + 8, :].rearrange(
                            "j w c -> j (w c)"), ps[:])

                def build_u2(cm):
                    u2 = u2p.tile([128, W, CHUNK], bf16, name="u2")
                    build_u2_octs(u2, cm, 0, 16)
                    return u2

                def build_zg(cm, i0):
                    # zg[j, d, c] = v[c0+c, i0+d, j] (row-pass lhsT)
                    zg = zp.tile([128, GI, CHUNK], bf16, name="zg")
                    if CONFIG["zg_mode"] == "pe":
                        ps = pst.tile([128, 1024], bf16, name="ps_t", tag="pst")
                        for d in range(GI):
                            nc.tensor.transpose(
                                ps[:, d * 128:(d + 1) * 128], cm[:, i0 + d, :], ident_bf[:])
                        evac_zg(zg[:].rearrange("j d c -> j (d c)"), ps[:])
                    else:
                        q = nc.sync if CONFIG["zg_mode"] == "xbar_sync" else nc.scalar
                        q.dma_start(zg[:], cm[:, i0:i0 + GI, :].rearrange(
                            "c a b -> c (a b)"), transpose=True)
                    return zg

                st_ct = [0]
                pending_stores = []

                def flush_store():
                    # stores are emitted one group late so their evac-done
                    # wait is already satisfied at queue head (a store that
                    # waits in a DMA queue convoys everything behind it).
                    # ACT is compute-loaded, so stores go sync/gpsimd only.
                    dst, src = pending_stores.pop(0)
                    qs = CONFIG["store_qs"]
                    q = getattr(nc, qs[st_ct[0] % len(qs)])
                    st_ct[0] += 1
                    q.dma_start(dst, src)

                # flat (chunk, group) schedule with 2-group zg lookahead
                sched = [(ck, g) for ck in range(N_CHUNKS) for g in range(NG)]
                zg_q = [build_zg(cm_tiles[0], a * GI) for a in range(CONFIG["zg_ahead"])]
                u2 = build_u2(cm_tiles[0])
                del build_u2

                for idx, (ck, g) in enumerate(sched):
                    cm = cm_tiles[ck]
                    i0 = g * GI
                    u2_pending = None
                    if g == 0:
                        # chunks 0/1 prefetched in phase 1; later loads rotate
                        # into the buffer freed by chunk ck-1
                        if ck >= 1 and ck + 1 < N_CHUNKS:
                            load_cmaj(ck + 1)
                        if ck > 0:
                            # first half of u2 now; second half interleaves
                            # after this group's rows so the PE bridges the
                            # evac latency before the cols need u2
                            u2 = u2p.tile([128, W, CHUNK], bf16, name="u2")
                            build_u2_octs(u2, cm, 0, 8)
                            u2_pending = cm

                    zg = zg_q.pop(0)
                    if idx + CONFIG["zg_ahead"] < len(sched):
                        nck, ng = sched[idx + CONFIG["zg_ahead"]]
                        zg_q.append(build_zg(cm_tiles[nck], ng * GI))

                    pg = pgp.tile([128, GI, W], f32, name="pg")
                    # rows: one zero-region start per psum bank
                    for d in range(GI):
                        i = i0 + d
                        nc.tensor.matmul(
                            pg[:, d, :], lhsT=zg[:, d, :],
                            rhs=att_W[:, i * W:(i + 1) * W],
                            start=(d % 4 == 0), stop=False,
                            skip_group_check=True,
                        )
                    if u2_pending is not None:
                        build_u2_octs(u2, u2_pending, 8, 16)
                    # cols: strided 4-i within-bank writes
                    for w in range(W):
                        for hb in range(2):
                            nc.tensor.matmul(
                                pg[:, hb * 4:hb * 4 + 4, w],
                                lhsT=u2[:, w, :],
                                rhs=att_H[:, w * H + i0 + hb * 4:
                                          w * H + i0 + hb * 4 + 4],
                                start=False,
                                stop=(w == W - 1),
                                skip_group_check=True,
                            )
                    # fused evac: out = pg * R (deferred softmax norm)
                    out_sb = outp.tile([128, GI, W], f32, name="out_sb")
                    nc.vector.tensor_tensor(
                        out_sb[:].rearrange("c a b -> c (a b)"),
                        pg[:].rearrange("c a b -> c (a b)"),
                        R_sb[:, i0 * W:(i0 + GI) * W], op=mult)
                    pending_stores.append((
                        o_d[ck * CHUNK:(ck + 1) * CHUNK, i0:i0 + GI, :],
                        out_sb[:],
                    ))
                    if len(pending_stores) > CONFIG.get("store_delay", 1):
                        flush_store()
                while pending_stores:
                    flush_store()

    nc.compile()
    return nc


_CACHE = {}
_LOCK = threading.Lock()


def _get_nc():
    with _LOCK:
        if "nc" not in _CACHE:
            _CACHE["nc"] = build_nc()
        return _CACHE["nc"]


def kernel(proj_query: np.ndarray, proj_key: np.ndarray, proj_value: np.ndarray,
           trace: bool = False):
    from concourse.bass_utils import run_bass_kernel_spmd

    q = np.ascontiguousarray(np.asarray(proj_query, dtype=np.float32))
    k = np.ascontiguousarray(np.asarray(proj_key, dtype=np.float32))
    v = np.ascontiguousarray(np.asarray(proj_value, dtype=np.float32))
    assert q.shape == (B, CQ, H, W) and v.shape == (B, CV, H, W)

    nc = _get_nc()
    in_maps = [{"q": q[b], "k": k[b], "v": v[b]} for b in range(B)]
    res = run_bass_kernel_spmd(nc, in_maps, core_ids=list(range(B)), trace=trace)
    out = np.stack([res.results[b]["o"] for b in range(B)], axis=0)
    if trace:
        kernel.last_exec_time_ns = res.exec_time_ns
        kernel.last_results = res
    return out


if __name__ == "__main__":
    nc = build_nc()
    print("build ok:", nc)
